# revision 12
# baseline (speedup 1.0000x reference)
import sys

if "/opt/trn_rl_repo" not in sys.path:
    sys.path.insert(0, "/opt/trn_rl_repo")

import ml_dtypes
import numpy as np

import concourse.bass as bass
import concourse.bacc as bacc
import concourse.tile as tile
from concourse import bass_utils, mybir
from concourse.alu_op_type import AluOpType

C = 8          # cores
G = 1024       # segments
SPC = G // C   # segments per core
D = 256        # feature dim
H = 128        # attention hidden dim
CHUNK = 1024   # nodes per pipeline chunk (8 tiles of 128)
TPC = CHUNK // 128  # node tiles per chunk
NSLOT = 6      # manual xn slot count
LAG = 4        # software-pipeline distance between scores and pooling

F32 = mybir.dt.float32
BF16 = mybir.dt.bfloat16

_cache: dict = {}


def _build(npad: int, b2val: float):
    nchunks = npad // CHUNK
    ntiles = npad // 128
    nc = bacc.Bacc("TRN2", target_bir_lowering=False, debug=False, num_devices=C)

    x_d = nc.dram_tensor("x", [npad, D], BF16, kind="ExternalInput")
    bloc_d = nc.dram_tensor("bloc", [128, ntiles], F32, kind="ExternalInput")
    w1a_d = nc.dram_tensor("w1a", [128, H], BF16, kind="ExternalInput")
    w1b_d = nc.dram_tensor("w1b", [128, H], BF16, kind="ExternalInput")
    w2_d = nc.dram_tensor("w2", [H, 1], BF16, kind="ExternalInput")
    b1_d = nc.dram_tensor("b1", [H, 1], F32, kind="ExternalInput")
    iota_d = nc.dram_tensor("iota", [128, SPC], F32, kind="ExternalInput")
    cnt_d = nc.dram_tensor("cnt", [SPC, 1], F32, kind="ExternalInput")
    o_d = nc.dram_tensor("o", [SPC, D], F32, kind="ExternalOutput")

    TANH = mybir.ActivationFunctionType.Tanh
    EXP = mybir.ActivationFunctionType.Exp

    with tile.TileContext(nc) as tc:
        with (
            tc.tile_pool(name="const", bufs=1) as constp,
            tc.tile_pool(name="xT", bufs=4) as xTp,
            tc.tile_pool(name="th", bufs=4) as thp,
            tc.tile_pool(name="eb", bufs=LAG + 4) as ebp,
            tc.tile_pool(name="sw", bufs=24) as swp,
            tc.tile_pool(name="fin", bufs=1) as finp,
            tc.tile_pool(name="ph", bufs=3, space="PSUM") as php,
            tc.tile_pool(name="ps", bufs=3, space="PSUM") as psp,
            tc.tile_pool(name="po", bufs=1, space="PSUM") as pop,
        ):
            w1a = constp.tile([128, H], BF16)
            nc.sync.dma_start(w1a[:], w1a_d[:])
            w1b = constp.tile([128, H], BF16)
            nc.sync.dma_start(w1b[:], w1b_d[:])
            w2 = constp.tile([H, 1], BF16)
            nc.sync.dma_start(w2[:], w2_d[:])
            b1 = constp.tile([H, 1], F32)
            nc.sync.dma_start(b1[:], b1_d[:])
            iota = constp.tile([128, SPC], F32)
            nc.sync.dma_start(iota[:], iota_d[:])
            cnt = constp.tile([SPC, 1], F32)
            nc.sync.dma_start(cnt[:], cnt_d[:])
            bloc = constp.tile([128, ntiles], F32)
            nc.sync.dma_start(bloc[:], bloc_d[:])

            # manual xn slots: natural-layout nodes plus a persistent ones column
            slots = []
            for s in range(NSLOT):
                sl = constp.tile([128, TPC, D + 1], BF16, tag=f"slot{s}")
                nc.gpsimd.memset(sl[:, :, D : D + 1], 1.0)
                slots.append(sl)

            psum_o = pop.tile([SPC, D + 1], F32)

            # softmax numerators for every node tile, written in phase 1,
            # read in phase 2 (phases are fully ordered, so one tile is fine)
            e_all = constp.tile([128, ntiles], F32)

            def scores(t):
                r0 = t * CHUNK
                # one transposed read per chunk; feature index = 2*p + h
                xT = xTp.tile([128, 2, CHUNK], BF16)
                nc.sync.dma_start_transpose(xT[:], x_d[r0 : r0 + CHUNK, :])

                th = thp.tile([H, CHUNK], BF16)
                for u in range(CHUNK // 512):
                    ph = php.tile([H, 512], F32)
                    nc.tensor.matmul(
                        ph[:], w1a[:], xT[:, 0, u * 512 : (u + 1) * 512],
                        start=True, stop=False,
                    )
                    nc.tensor.matmul(
                        ph[:], w1b[:], xT[:, 1, u * 512 : (u + 1) * 512],
                        start=False, stop=True,
                    )
                    nc.scalar.activation(
                        th[:, u * 512 : (u + 1) * 512], ph[:], TANH,
                        bias=b1[:], scale=1.0,
                    )

                ps = psp.tile([128, TPC], F32)
                for j in range(TPC):
                    nc.tensor.matmul(
                        ps[:, j : j + 1],
                        th[:, j * 128 : (j + 1) * 128],
                        w2[:],
                        start=True,
                        stop=True,
                    )
                nc.scalar.activation(
                    e_all[:, t * TPC : (t + 1) * TPC], ps[:], EXP,
                    bias=b2val, scale=1.0,
                )

            def pool(t):
                r0 = t * CHUNK
                # one natural read per chunk into slot (row a*128+p -> [p, a, :])
                xn = slots[t % NSLOT]
                nc.sync.dma_start(
                    xn[:, :, 0:D],
                    x_d[r0 : r0 + CHUNK, :].rearrange("(a p) d -> p a d", p=128),
                )
                for j in range(TPC):
                    g = t * TPC + j
                    sw = swp.tile([128, SPC], BF16)
                    nc.vector.tensor_scalar(
                        sw[:],
                        iota[:],
                        bloc[:, g : g + 1],
                        e_all[:, g : g + 1],
                        AluOpType.is_equal,
                        AluOpType.mult,
                    )
                    nc.tensor.matmul(
                        psum_o[:],
                        sw[:],
                        xn[:, j, :],
                        start=(t == 0 and j == 0),
                        stop=(t == nchunks - 1 and j == TPC - 1),
                    )

            for t in range(nchunks):
                scores(t)
            for t in range(nchunks):
                pool(t)

            dent = finp.tile([SPC, 1], F32)
            nc.vector.tensor_scalar(
                dent[:],
                psum_o[:, D : D + 1],
                cnt[:],
                1e-30,
                AluOpType.mult,
                AluOpType.max,
            )
            rec = finp.tile([SPC, 1], F32)
            nc.vector.reciprocal(rec[:], dent[:])
            osb = finp.tile([SPC, D], F32)
            nc.vector.tensor_scalar_mul(osb[:], psum_o[:, 0:D], rec[:])
            nc.sync.dma_start(o_d[:], osb[:])

    nc.compile()
    return nc


def kernel(x, batch, W1, b1, W2, b2):
    x = np.asarray(x)
    batch = np.asarray(batch)
    W1 = np.asarray(W1, np.float32)
    b1 = np.asarray(b1, np.float32)
    W2 = np.asarray(W2, np.float32)
    b2 = np.asarray(b2, np.float32)

    bat = batch.astype(np.int64)
    # per-core node ranges: core c owns segments [c*SPC, (c+1)*SPC)
    bounds = np.searchsorted(bat, np.arange(0, G + 1, SPC), side="left")
    ncounts = np.diff(bounds)
    npad = int(-(-ncounts.max() // CHUNK) * CHUNK)
    ntiles = npad // 128

    counts = np.bincount(bat, minlength=G).astype(np.float32)

    key = (npad, float(b2[0]))
    if key not in _cache:
        _cache[key] = _build(npad, float(b2[0]))
    nc = _cache[key]

    x_bf = x.astype(ml_dtypes.bfloat16)
    # 3D dma-transpose layout: xT[p, h, n] = x[n, 128*h + p] (probed on hw),
    # so W1 halves stay in natural order
    w1a = W1[0:128, :].astype(ml_dtypes.bfloat16)
    w1b = W1[128:256, :].astype(ml_dtypes.bfloat16)
    w2 = W2.reshape(H, 1).astype(ml_dtypes.bfloat16)
    b1c = b1.reshape(H, 1).astype(np.float32)
    iota = np.broadcast_to(
        np.arange(SPC, dtype=np.float32)[None, :], (128, SPC)
    ).copy()

    in_maps = []
    for c in range(C):
        s, e = bounds[c], bounds[c + 1]
        nct = e - s
        xc = np.zeros((npad, D), ml_dtypes.bfloat16)
        xc[:nct] = x_bf[s:e]
        blc = np.full((npad,), -1.0, np.float32)
        blc[:nct] = (bat[s:e] - c * SPC).astype(np.float32)
        # [p, tile] layout: col t*TPC+j, row p = node (t*TPC+j)*128 + p
        blc = np.ascontiguousarray(blc.reshape(ntiles, 128).T)
        cntc = np.maximum(counts[c * SPC : (c + 1) * SPC], 1.0).reshape(SPC, 1)
        in_maps.append(
            {
                "x": xc,
                "bloc": blc,
                "w1a": w1a,
                "w1b": w1b,
                "w2": w2,
                "b1": b1c,
                "iota": iota,
                "cnt": cntc,
            }
        )

    res = bass_utils.run_bass_kernel_spmd(nc, in_maps, core_ids=list(range(C)))
    out = np.concatenate([res.results[c]["o"] for c in range(C)], axis=0)
    return out.astype(np.float32)


# revision 19
# speedup vs baseline: 33110.0538x; 33110.0538x over previous
import sys

if "/opt/trn_rl_repo" not in sys.path:
    sys.path.insert(0, "/opt/trn_rl_repo")

import ml_dtypes
import numpy as np

import concourse.bass as bass
import concourse.bacc as bacc
import concourse.tile as tile
from concourse import bass_utils, mybir
from concourse.alu_op_type import AluOpType

C = 8          # cores
G = 1024       # segments
SPC = G // C   # segments per core
D = 256        # feature dim
H = 128        # attention hidden dim
CHUNK = 1024   # nodes per pipeline chunk (8 tiles of 128)
TPC = CHUNK // 128  # node tiles per chunk
NSLOT = 8      # manual xn slot count
LAG = 4        # software-pipeline distance between scores and pooling

F32 = mybir.dt.float32
BF16 = mybir.dt.bfloat16

_cache: dict = {}


def _build(npad: int, b2val: float, repeat: int = 1):
    nchunks = npad // CHUNK
    ntiles = npad // 128
    nc = bacc.Bacc("TRN2", target_bir_lowering=False, debug=False, num_devices=C)

    x_d = nc.dram_tensor("x", [npad, D], BF16, kind="ExternalInput")
    bloc_d = nc.dram_tensor("bloc", [128, ntiles], F32, kind="ExternalInput")
    w1a_d = nc.dram_tensor("w1a", [128, H], BF16, kind="ExternalInput")
    w1b_d = nc.dram_tensor("w1b", [128, H], BF16, kind="ExternalInput")
    w2_d = nc.dram_tensor("w2", [H, 1], BF16, kind="ExternalInput")
    b1_d = nc.dram_tensor("b1", [H, 1], F32, kind="ExternalInput")
    iota_d = nc.dram_tensor("iota", [128, SPC], F32, kind="ExternalInput")
    cnt_d = nc.dram_tensor("cnt", [SPC, 1], F32, kind="ExternalInput")
    o_d = nc.dram_tensor("o", [SPC, D], F32, kind="ExternalOutput")

    TANH = mybir.ActivationFunctionType.Tanh
    EXP = mybir.ActivationFunctionType.Exp

    with tile.TileContext(nc) as tc:
        with (
            tc.tile_pool(name="const", bufs=1) as constp,
            tc.tile_pool(name="xT", bufs=6) as xTp,
            tc.tile_pool(name="th", bufs=6) as thp,
            tc.tile_pool(name="eb", bufs=LAG + 4) as ebp,
            tc.tile_pool(name="sw", bufs=24) as swp,
            tc.tile_pool(name="fin", bufs=1) as finp,
            tc.tile_pool(name="ph", bufs=3, space="PSUM") as php,
            tc.tile_pool(name="ps", bufs=3, space="PSUM") as psp,
            tc.tile_pool(name="po", bufs=1, space="PSUM") as pop,
        ):
            w1a = constp.tile([128, H], BF16)
            nc.sync.dma_start(w1a[:], w1a_d[:])
            w1b = constp.tile([128, H], BF16)
            nc.sync.dma_start(w1b[:], w1b_d[:])
            w2 = constp.tile([H, 1], BF16)
            nc.sync.dma_start(w2[:], w2_d[:])
            b1 = constp.tile([H, 1], F32)
            nc.sync.dma_start(b1[:], b1_d[:])
            iota = constp.tile([128, SPC], F32)
            nc.sync.dma_start(iota[:], iota_d[:])
            cnt = constp.tile([SPC, 1], F32)
            nc.sync.dma_start(cnt[:], cnt_d[:])
            bloc = constp.tile([128, ntiles], F32)
            nc.sync.dma_start(bloc[:], bloc_d[:])

            # manual xn slots: natural-layout nodes plus a persistent ones column
            slots = []
            for s in range(NSLOT):
                sl = constp.tile([128, TPC, D + 1], BF16, tag=f"slot{s}")
                nc.gpsimd.memset(sl[:, :, D : D + 1], 1.0)
                slots.append(sl)

            psum_o = pop.tile([SPC, D + 1], F32)

            # softmax numerators for every node tile, written in phase 1,
            # read in phase 2 (phases are fully ordered, so one tile is fine)
            e_all = constp.tile([128, ntiles], F32)

            def scores(t):
                r0 = t * CHUNK
                # one transposed read per chunk; feature index = 2*p + h
                xT = xTp.tile([128, 2, CHUNK], BF16)
                nc.sync.dma_start_transpose(xT[:], x_d[r0 : r0 + CHUNK, :])

                th = thp.tile([H, CHUNK], BF16)
                for u in range(CHUNK // 512):
                    ph = php.tile([H, 512], F32)
                    nc.tensor.matmul(
                        ph[:], w1a[:], xT[:, 0, u * 512 : (u + 1) * 512],
                        start=True, stop=False,
                    )
                    nc.tensor.matmul(
                        ph[:], w1b[:], xT[:, 1, u * 512 : (u + 1) * 512],
                        start=False, stop=True,
                    )
                    nc.scalar.activation(
                        th[:, u * 512 : (u + 1) * 512], ph[:], TANH,
                        bias=b1[:], scale=1.0,
                    )

                ps = psp.tile([128, TPC], F32)
                for j in range(TPC):
                    nc.tensor.matmul(
                        ps[:, j : j + 1],
                        th[:, j * 128 : (j + 1) * 128],
                        w2[:],
                        start=True,
                        stop=True,
                    )
                nc.scalar.activation(
                    e_all[:, t * TPC : (t + 1) * TPC], ps[:], EXP,
                    bias=b2val, scale=1.0,
                )

            def pool(t, first=True, last=True):
                r0 = t * CHUNK
                # one natural read per chunk into slot (row a*128+p -> [p, a, :])
                xn = slots[t % NSLOT]
                nc.sync.dma_start(
                    xn[:, :, 0:D],
                    x_d[r0 : r0 + CHUNK, :].rearrange("(a p) d -> p a d", p=128),
                )
                for j in range(TPC):
                    g = t * TPC + j
                    sw = swp.tile([128, SPC], BF16)
                    nc.vector.tensor_scalar(
                        sw[:],
                        iota[:],
                        bloc[:, g : g + 1],
                        e_all[:, g : g + 1],
                        AluOpType.is_equal,
                        AluOpType.mult,
                    )
                    nc.tensor.matmul(
                        psum_o[:],
                        sw[:],
                        xn[:, j, :],
                        start=(first and t == 0 and j == 0),
                        stop=(last and t == nchunks - 1 and j == TPC - 1),
                    )

            for rep in range(repeat):
                for t in range(nchunks):
                    scores(t)
                for t in range(nchunks):
                    pool(t, first=(rep == 0), last=(rep == repeat - 1))

            dent = finp.tile([SPC, 1], F32)
            nc.vector.tensor_scalar(
                dent[:],
                psum_o[:, D : D + 1],
                cnt[:],
                1e-30,
                AluOpType.mult,
                AluOpType.max,
            )
            rec = finp.tile([SPC, 1], F32)
            nc.vector.reciprocal(rec[:], dent[:])
            osb = finp.tile([SPC, D], F32)
            nc.vector.tensor_scalar_mul(osb[:], psum_o[:, 0:D], rec[:])
            nc.sync.dma_start(o_d[:], osb[:])

    nc.compile()
    return nc


def kernel(x, batch, W1, b1, W2, b2):
    x = np.asarray(x)
    batch = np.asarray(batch)
    W1 = np.asarray(W1, np.float32)
    b1 = np.asarray(b1, np.float32)
    W2 = np.asarray(W2, np.float32)
    b2 = np.asarray(b2, np.float32)

    bat = batch.astype(np.int64)
    # per-core node ranges: core c owns segments [c*SPC, (c+1)*SPC)
    bounds = np.searchsorted(bat, np.arange(0, G + 1, SPC), side="left")
    ncounts = np.diff(bounds)
    npad = int(-(-ncounts.max() // CHUNK) * CHUNK)
    ntiles = npad // 128

    counts = np.bincount(bat, minlength=G).astype(np.float32)

    key = (npad, float(b2[0]))
    if key not in _cache:
        _cache[key] = _build(npad, float(b2[0]))
    nc = _cache[key]

    x_bf = x.astype(ml_dtypes.bfloat16)
    # 3D dma-transpose layout: xT[p, h, n] = x[n, 128*h + p] (probed on hw),
    # so W1 halves stay in natural order
    w1a = W1[0:128, :].astype(ml_dtypes.bfloat16)
    w1b = W1[128:256, :].astype(ml_dtypes.bfloat16)
    w2 = W2.reshape(H, 1).astype(ml_dtypes.bfloat16)
    b1c = b1.reshape(H, 1).astype(np.float32)
    iota = np.broadcast_to(
        np.arange(SPC, dtype=np.float32)[None, :], (128, SPC)
    ).copy()

    in_maps = []
    for c in range(C):
        s, e = bounds[c], bounds[c + 1]
        nct = e - s
        xc = np.zeros((npad, D), ml_dtypes.bfloat16)
        xc[:nct] = x_bf[s:e]
        blc = np.full((npad,), -1.0, np.float32)
        blc[:nct] = (bat[s:e] - c * SPC).astype(np.float32)
        # [p, tile] layout: col t*TPC+j, row p = node (t*TPC+j)*128 + p
        blc = np.ascontiguousarray(blc.reshape(ntiles, 128).T)
        cntc = np.maximum(counts[c * SPC : (c + 1) * SPC], 1.0).reshape(SPC, 1)
        in_maps.append(
            {
                "x": xc,
                "bloc": blc,
                "w1a": w1a,
                "w1b": w1b,
                "w2": w2,
                "b1": b1c,
                "iota": iota,
                "cnt": cntc,
            }
        )

    res = bass_utils.run_bass_kernel_spmd(nc, in_maps, core_ids=list(range(C)))
    out = np.concatenate([res.results[c]["o"] for c in range(C)], axis=0)
    return out.astype(np.float32)
